# revision 1
# baseline (speedup 1.0000x reference)
"""Associative-embedding (push/pull) loss on 8 TRN2 NeuronCores.

Strategy (data parallel, 8 images per core):
  - The 285MB tags tensor is only ever touched at P*K=510 points per image,
    so instead of streaming it, each core does ONE indirect DMA gather of
    30x136 f32 elements with on-device computed flat indices
    idx = 65536*(img*17+k) + 256*x + y.
  - All per-person masked moments are computed in a [30 part, (img,k)] layout
    on DVE; person-axis reductions and the pairwise push term run in an
    [img part, person] layout after a 32x32 DVE block transpose.
  - Each core emits (sum_push, sum_pull)/64 partials; cores 0..7 partials are
    summed (optionally via on-device AllReduce) to form the final two scalars.
"""

import os
import sys

import numpy as np

if "/opt/trn_rl_repo" not in sys.path:
    sys.path.insert(0, "/opt/trn_rl_repo")

from concourse import bacc, bass, mybir, tile  # noqa: E402
from concourse import bass_utils  # noqa: E402

B, P, K, H, W = 64, 30, 17, 256, 256
NCORES = 8
BPC = B // NCORES           # 8 images per core
J = BPC * K                 # 136 (img, k) columns
KHW = K * H * W             # 1114112
NTOT = BPC * KHW

f32 = mybir.dt.float32
i32 = mybir.dt.int32
Alu = mybir.AluOpType
Act = mybir.ActivationFunctionType
AX = mybir.AxisListType


def build_nc(collective: bool = False):
    nc = bacc.Bacc("TRN2", target_bir_lowering=False, debug=False,
                   num_devices=NCORES)

    tags = nc.dram_tensor("tags", [NTOT + 64, 1], f32, kind="ExternalInput")
    joints = nc.dram_tensor("joints", [BPC, P, K, 2], i32, kind="ExternalInput")
    jv = nc.dram_tensor("jv", [BPC, P, K], i32, kind="ExternalInput")
    pv = nc.dram_tensor("pv", [BPC, P], i32, kind="ExternalInput")
    out = nc.dram_tensor("out", [2, 1], f32, kind="ExternalOutput")

    with tile.TileContext(nc) as tc:
        with tc.tile_pool(name="sbuf", bufs=1) as pool, \
             tc.tile_pool(name="psum", bufs=1, space="PSUM") as psp:

            # ---- load the small tensors, transposed to [p, (img,k)] ----
            jnt = pool.tile([P, J, 2], i32)
            jvt = pool.tile([P, J], i32)
            pvt = pool.tile([P, BPC], i32)
            nc.sync.dma_start(out=jnt[:], in_=joints[:].rearrange("b p k c -> p b k c"))
            nc.sync.dma_start(out=jvt[:], in_=jv[:].rearrange("b p k -> p b k"))
            nc.sync.dma_start(out=pvt[:], in_=pv[:].rearrange("b p -> p b"))

            # ---- gather indices: idx = 65536*(img*17+k) + 256*x + y ----
            ibase = pool.tile([P, J], i32)
            nc.gpsimd.iota(ibase[:], pattern=[[1, J]], base=0,
                           channel_multiplier=0)
            idx = pool.tile([P, J], i32)
            nc.vector.tensor_scalar(out=idx[:], in0=jnt[:, :, 0], scalar1=256,
                                    scalar2=None, op0=Alu.mult)
            nc.vector.tensor_tensor(out=idx[:], in0=idx[:], in1=jnt[:, :, 1],
                                    op=Alu.add)
            # idx += 65536 * ibase
            nc.vector.scalar_tensor_tensor(out=idx[:], in0=ibase[:],
                                           scalar=65536, in1=idx[:],
                                           op0=Alu.mult, op1=Alu.add)

            # ---- gather: vals[p, (img,k)] = tags_flat[idx] ----
            # HW indirect DMA contract (per-partition): out[q, :] =
            # in_flat[idx[q, 0] : idx[q, 0] + free_size].  So issue NCALL
            # calls of one 4-elem window per partition, with the 30x136
            # index matrix re-tiled onto 120 partitions (4 blocks of 30).
            NCALL = J // 4          # 34
            xs = pool.tile([128, NCALL], i32)
            nc.vector.memset(xs[:], 0)
            for a in range(4):
                nc.vector.tensor_copy(
                    out=xs[a * 32:a * 32 + P, :],
                    in_=idx[:, a * NCALL:(a + 1) * NCALL])
            v4 = pool.tile([128, NCALL, 4], f32)
            for c in range(NCALL):
                nc.gpsimd.indirect_dma_start(
                    out=v4[:, c, :], out_offset=None, in_=tags[:],
                    in_offset=bass.IndirectOffsetOnAxis(ap=xs[:, c:c + 1],
                                                        axis=0))
            vals = pool.tile([P, J], f32)
            for a in range(4):
                nc.vector.tensor_copy(
                    out=vals[:, a * NCALL:(a + 1) * NCALL],
                    in_=v4[a * 32:a * 32 + P, :, 0])

            # ---- masks + masked moments over k ----
            mjv = pool.tile([P, J], f32)
            mpv = pool.tile([P, BPC], f32)
            nc.vector.tensor_scalar(out=mjv[:], in0=jvt[:], scalar1=0,
                                    scalar2=None, op0=Alu.is_gt)
            nc.vector.tensor_scalar(out=mpv[:], in0=pvt[:], scalar1=0,
                                    scalar2=None, op0=Alu.is_gt)
            m = pool.tile([P, BPC, K], f32)
            nc.vector.tensor_tensor(
                out=m[:], in0=mjv[:].rearrange("p (b k) -> p b k", k=K),
                in1=mpv[:].unsqueeze(2).to_broadcast([P, BPC, K]), op=Alu.mult)

            mv = pool.tile([P, J], f32)
            mv2 = pool.tile([P, J], f32)
            m_flat = m[:].rearrange("p b k -> p (b k)")
            nc.vector.tensor_tensor(out=mv[:], in0=m_flat, in1=vals[:],
                                    op=Alu.mult)
            nc.vector.tensor_tensor(out=mv2[:], in0=mv[:], in1=vals[:],
                                    op=Alu.mult)

            cnt = pool.tile([P, BPC], f32)
            s1 = pool.tile([P, BPC], f32)
            s2 = pool.tile([P, BPC], f32)
            nc.vector.tensor_reduce(out=cnt[:], in_=m[:], axis=AX.X, op=Alu.add)
            nc.vector.tensor_reduce(out=s1[:],
                                    in_=mv[:].rearrange("p (b k) -> p b k", k=K),
                                    axis=AX.X, op=Alu.add)
            nc.vector.tensor_reduce(out=s2[:],
                                    in_=mv2[:].rearrange("p (b k) -> p b k", k=K),
                                    axis=AX.X, op=Alu.add)

            safe = pool.tile([P, BPC], f32)
            inv = pool.tile([P, BPC], f32)
            nc.vector.tensor_scalar(out=safe[:], in0=cnt[:], scalar1=1.0,
                                    scalar2=None, op0=Alu.max)
            nc.vector.reciprocal(out=inv[:], in_=safe[:])

            # staging tile for the 32x32 block transpose:
            # block0 = mean, block1 = pm, block2 = pull_p
            t_in = pool.tile([32, 96], f32)
            t_out = pool.tile([32, 96], f32)
            nc.vector.memset(t_in[:], 0.0)
            mean = t_in[0:P, 0:BPC]
            pm = t_in[0:P, 32:32 + BPC]
            pullp = t_in[0:P, 64:64 + BPC]
            nc.vector.tensor_tensor(out=mean, in0=s1[:], in1=inv[:], op=Alu.mult)
            nc.vector.tensor_scalar(out=pm, in0=cnt[:], scalar1=0.0,
                                    scalar2=None, op0=Alu.is_gt)
            # pull_p = (s2 - s1*mean) * inv
            tmp = pool.tile([P, BPC], f32)
            nc.vector.tensor_tensor(out=tmp[:], in0=s1[:], in1=mean, op=Alu.mult)
            nc.vector.tensor_tensor(out=tmp[:], in0=s2[:], in1=tmp[:],
                                    op=Alu.subtract)
            nc.vector.tensor_tensor(out=pullp, in0=tmp[:], in1=inv[:],
                                    op=Alu.mult)

            nc.vector.transpose(out=t_out[:], in_=t_in[:])
            meanT = t_out[0:BPC, 0:P]          # [8, 30]
            pmT = t_out[0:BPC, 32:32 + P]
            pullT = t_out[0:BPC, 64:64 + P]

            # ---- push: pairwise exp(-(mean_p - mean_q)^2) ----
            d2 = pool.tile([BPC, P, P], f32)
            nc.vector.tensor_tensor(
                out=d2[:],
                in0=meanT.unsqueeze(2).to_broadcast([BPC, P, P]),
                in1=meanT.unsqueeze(1).to_broadcast([BPC, P, P]),
                op=Alu.subtract)
            nc.vector.tensor_tensor(out=d2[:], in0=d2[:], in1=d2[:],
                                    op=Alu.mult)
            e = pool.tile([BPC, P, P], f32)
            nc.scalar.activation(out=e[:], in_=d2[:], func=Act.Exp, scale=-1.0)
            pair = pool.tile([BPC, P, P], f32)
            nc.vector.tensor_tensor(
                out=pair[:],
                in0=pmT.unsqueeze(2).to_broadcast([BPC, P, P]),
                in1=pmT.unsqueeze(1).to_broadcast([BPC, P, P]),
                op=Alu.mult)
            ep = pool.tile([BPC, P, P], f32)
            s_img = pool.tile([BPC, 1], f32)
            nc.vector.tensor_tensor(out=ep[:], in0=e[:], in1=pair[:],
                                    op=Alu.mult)
            nc.vector.tensor_reduce(out=s_img[:], in_=ep[:], axis=AX.XY,
                                    op=Alu.add)

            n_img = pool.tile([BPC, 1], f32)
            pull_sum = pool.tile([BPC, 1], f32)
            nc.vector.tensor_reduce(out=n_img[:], in_=pmT, axis=AX.X, op=Alu.add)
            nc.vector.tensor_reduce(out=pull_sum[:], in_=pullT, axis=AX.X,
                                    op=Alu.add)

            # push_img = (s - n) * 0.5 / max(n^2-n, 1) * (n > 1)
            pp2 = pool.tile([BPC, 2], f32)
            den = pool.tile([BPC, 1], f32)
            dinv = pool.tile([BPC, 1], f32)
            g = pool.tile([BPC, 1], f32)
            u = pool.tile([BPC, 1], f32)
            nc.vector.tensor_tensor(out=den[:], in0=n_img[:], in1=n_img[:],
                                    op=Alu.mult)
            nc.vector.tensor_tensor(out=den[:], in0=den[:], in1=n_img[:],
                                    op=Alu.subtract)
            nc.vector.tensor_scalar(out=den[:], in0=den[:], scalar1=1.0,
                                    scalar2=None, op0=Alu.max)
            nc.vector.reciprocal(out=dinv[:], in_=den[:])
            nc.vector.tensor_scalar(out=g[:], in0=n_img[:], scalar1=1.0,
                                    scalar2=None, op0=Alu.is_gt)
            nc.vector.tensor_tensor(out=u[:], in0=s_img[:], in1=n_img[:],
                                    op=Alu.subtract)
            nc.vector.tensor_tensor(out=u[:], in0=u[:], in1=dinv[:], op=Alu.mult)
            nc.vector.tensor_tensor(out=u[:], in0=u[:], in1=g[:], op=Alu.mult)
            nc.vector.tensor_scalar(out=pp2[:, 0:1], in0=u[:], scalar1=0.5,
                                    scalar2=None, op0=Alu.mult)

            # pull_img = pull_sum / max(n, 1)
            nm = pool.tile([BPC, 1], f32)
            ninv = pool.tile([BPC, 1], f32)
            nc.vector.tensor_scalar(out=nm[:], in0=n_img[:], scalar1=1.0,
                                    scalar2=None, op0=Alu.max)
            nc.vector.reciprocal(out=ninv[:], in_=nm[:])
            nc.vector.tensor_tensor(out=pp2[:, 1:2], in0=pull_sum[:],
                                    in1=ninv[:], op=Alu.mult)

            # ---- sum over the 8 local images: psum[2,1] = pp2^T @ ones ----
            ones8 = pool.tile([BPC, 1], f32)
            nc.vector.memset(ones8[:], 1.0)
            acc = psp.tile([2, 1], f32)
            nc.tensor.matmul(out=acc[:], lhsT=pp2[:], rhs=ones8[:],
                             start=True, stop=True)
            res = pool.tile([2, 1], f32)
            nc.scalar.activation(out=res[:], in_=acc[:], func=Act.Copy,
                                 scale=1.0 / B)

            if collective:
                with tc.tile_pool(name="dram", bufs=1, space="DRAM") as dram:
                    ar_in = dram.tile([2, 1], f32)
                    ar_out = dram.tile([2, 1], f32)
                    nc.sync.dma_start(out=ar_in[:], in_=res[:])
                    nc.gpsimd.collective_compute(
                        "AllReduce", Alu.add,
                        replica_groups=[list(range(NCORES))],
                        ins=[ar_in.opt()],
                        outs=[ar_out.opt()],
                    )
                    nc.sync.dma_start(out=out[:], in_=ar_out[:])
            else:
                nc.sync.dma_start(out=out[:], in_=res[:])

    nc.compile()
    return nc


_nc_cache = {}


def _get_nc(collective: bool):
    if collective not in _nc_cache:
        _nc_cache[collective] = build_nc(collective)
    return _nc_cache[collective]


def _pad_tags(shard):
    buf = np.zeros((NTOT + 64, 1), dtype=np.float32)
    buf[:NTOT, 0] = shard.reshape(-1)
    return buf


def kernel(tags, joints, joint_img_valid, person_valid):
    collective = os.environ.get("AELOSS_COLLECTIVE", "0") == "1"
    nc = _get_nc(collective)

    tags = np.ascontiguousarray(np.asarray(tags, dtype=np.float32))
    joints = np.ascontiguousarray(np.asarray(joints, dtype=np.int32))
    jv = np.ascontiguousarray(np.asarray(joint_img_valid, dtype=np.int32))
    pv = np.ascontiguousarray(np.asarray(person_valid, dtype=np.int32))

    in_maps = []
    for c in range(NCORES):
        sl = slice(c * BPC, (c + 1) * BPC)
        in_maps.append({
            "tags": _pad_tags(tags[sl]),
            "joints": joints[sl],
            "jv": jv[sl],
            "pv": pv[sl],
        })

    res = bass_utils.run_bass_kernel_spmd(nc, in_maps,
                                          core_ids=list(range(NCORES)))
    outs = [np.asarray(r["out"], dtype=np.float64).reshape(2)
            for r in res.results]
    if collective:
        total = outs[0]
    else:
        total = np.sum(outs, axis=0)
    return np.float32(total[0]), np.float32(total[1])


if __name__ == "__main__":
    rng = np.random.default_rng(0)
    t = rng.standard_normal((B, K, H, W), dtype=np.float32)
    j = rng.integers(0, H, size=(B, P, K, 2), dtype=np.int32)
    jv_ = rng.integers(0, 2, size=(B, P, K), dtype=np.int32)
    pv_ = rng.integers(0, 2, size=(B, P), dtype=np.int32)
    print(kernel(t, j, jv_, pv_))

